# revision 4
# baseline (speedup 1.0000x reference)
"""CrossModalAttentionScorer Trainium2 kernel (Bass/Tile, 8 NeuronCores).

Reference computation (per batch b):
    R = anchor @ W_region            [A, H]
    Q = query  @ W_query             [T, H]
    S = R @ Q.T  (masked over T)     [A, T]
    P = softmax(S, axis=T)
    att = P @ Q                      [A, H]
    out = relu(concat([anchor, att, anchor*att]) @ W_combine + b)   [A, H]

Shapes: B=32, A=1024, T=64, D=H=512.

Sharding: data-parallel over B across 8 cores (4 batches/core), weights
replicated. Host pre-transposes anchor -> [B, D, A] and query -> [B, D, T] so
the contraction dim is the SBUF partition dim everywhere; every intermediate
is kept feature-major ("transposed") on chip, which makes all five matmuls
chain with no on-device input transposes. Only the [A,T] attention matrix is
PE-transposed (tiny). Matmuls run in float32r (TF32-like, ~1.5e-4 rel err,
4x the fp32 rate at N=512); PSUM accumulation is fp32. The query mask and
the combine bias are folded into the PSUM accumulation as rank-1 (K=1)
matmuls, so masking/bias costs no vector work.

scores_fp32=True falls back to full-fp32 matmuls for the pre-softmax path
(anchor/query projections + scores) at ~4x their PE cost, if the f32r
rounding of attention logits ever proves too coarse.
"""
import numpy as np

import concourse.bacc as bacc
import concourse.tile as tile
import concourse.mybir as mybir
from concourse.bass_utils import run_bass_kernel_spmd
from concourse.masks import make_identity

B, A, T, D, H = 32, 1024, 64, 512, 512
NCORES = 8
PB = B // NCORES          # batches per core = 4
P = 128                   # partitions
DT = D // P               # 4 d-tiles
HT = H // P               # 4 h-tiles
CT = 3 * H // P           # 12 c-tiles (concat dim)
ACH = 512                 # a-chunk (moving-dim) size
NCH = A // ACH            # 2 chunks per batch
AT_CH = ACH // P          # 4 a-tiles per chunk

F32 = mybir.dt.float32
F32R = mybir.dt.float32r
AX = mybir.AxisListType.X
AFT = mybir.ActivationFunctionType

_CACHE = {}


def build(scores_fp32: bool = False):
    """Build the per-core Bass module (4 batches of the problem)."""
    nc = bacc.Bacc(None, target_bir_lowering=False, debug=False)
    sdt = F32 if scores_fp32 else F32R  # dtype of the pre-softmax path

    aT = nc.dram_tensor("aT", [PB, D, A], sdt, kind="ExternalInput")
    qT = nc.dram_tensor("qT", [PB, D, T], sdt, kind="ExternalInput")
    mb = nc.dram_tensor("mb", [PB, 1, T], sdt, kind="ExternalInput")
    wr = nc.dram_tensor("wr", [D, H], sdt, kind="ExternalInput")
    wq = nc.dram_tensor("wq", [D, H], sdt, kind="ExternalInput")
    wc = nc.dram_tensor("wc", [3 * H, H], F32R, kind="ExternalInput")
    bc = nc.dram_tensor("bc", [1, H], F32R, kind="ExternalInput")
    x = nc.dram_tensor("x", [PB, A, H], F32, kind="ExternalOutput")

    with tile.TileContext(nc) as tc:
        with (
            tc.tile_pool(name="const", bufs=1) as const,
            tc.tile_pool(name="perb", bufs=2) as perb,
            tc.tile_pool(name="chunk", bufs=2) as chunk,
            tc.tile_pool(name="small", bufs=4) as small,
            tc.tile_pool(name="stage", bufs=3) as stage,
            tc.tile_pool(name="psum", bufs=1, space="PSUM") as psum,
        ):
            # ---- constants ----
            wr_sb, wq_sb, wc_sb = [], [], []
            for d in range(DT):
                t = const.tile([P, H], sdt, name=f"wr{d}")
                nc.sync.dma_start(out=t, in_=wr[d * P:(d + 1) * P, :])
                wr_sb.append(t)
                t = const.tile([P, H], sdt, name=f"wq{d}")
                nc.sync.dma_start(out=t, in_=wq[d * P:(d + 1) * P, :])
                wq_sb.append(t)
            for k in range(CT):
                t = const.tile([P, H], F32R, name=f"wc{k}")
                nc.sync.dma_start(out=t, in_=wc[k * P:(k + 1) * P, :])
                wc_sb.append(t)
            bc_sb = const.tile([1, H], F32R, name="bc_sb")
            nc.sync.dma_start(out=bc_sb, in_=bc[:, :])
            ones_f = const.tile([1, P], F32, name="ones_f")
            nc.vector.memset(ones_f, 1.0)
            ones_r = const.tile([1, P], F32R, name="ones_r")
            nc.vector.tensor_copy(ones_r[:], ones_f[:])
            ones_s = ones_f if scores_fp32 else ones_r
            ident = const.tile([P, P], F32, name="ident")
            make_identity(nc, ident)
            ident_r = const.tile([P, P], F32R, name="ident_r")
            nc.vector.tensor_copy(ident_r[:], ident[:])

            for i in range(PB):
                # ---- per-batch: query projections (tiny) ----
                qT_sb = []
                for d in range(DT):
                    t = perb.tile([P, T], sdt, tag=f"qT{d}", name=f"qT{d}")
                    nc.sync.dma_start(out=t, in_=qT[i, d * P:(d + 1) * P, :])
                    qT_sb.append(t)
                mb_sb = perb.tile([1, T], sdt, tag="mb", name="mb_sb")
                nc.sync.dma_start(out=mb_sb, in_=mb[i, :, :])

                # Q natural [T, H] (lhsT for attendedT)
                ps_q = psum.tile([T, H], F32, tag="mm512", name="ps_q")
                for d in range(DT):
                    nc.tensor.matmul(ps_q[:], qT_sb[d][:], wq_sb[d][:],
                                     start=(d == 0), stop=(d == DT - 1))
                qn_sb = perb.tile([T, H], F32R, tag="qn", name="qn_sb")
                nc.scalar.activation(qn_sb[:], ps_q[:], AFT.Copy)

                # Q^T [H, T] (rhs for scores), computed directly
                qt_sb = []
                for h in range(HT):
                    ps_qt = psum.tile([P, T], F32, tag="sc", name="ps_qt")
                    for d in range(DT):
                        nc.tensor.matmul(
                            ps_qt[:], wq_sb[d][:, h * P:(h + 1) * P], qT_sb[d][:],
                            start=(d == 0), stop=(d == DT - 1))
                    t = perb.tile([P, T], sdt, tag=f"qt{h}", name=f"qt{h}")
                    nc.vector.tensor_copy(t[:], ps_qt[:])
                    qt_sb.append(t)

                for c in range(NCH):
                    asl = slice(c * ACH, (c + 1) * ACH)
                    # ---- anchor^T chunk [D, ACH] ----
                    aT_sb = []
                    for d in range(DT):
                        t = chunk.tile([P, ACH], sdt, tag=f"aT{d}", name=f"aT{d}")
                        nc.sync.dma_start(out=t, in_=aT[i, d * P:(d + 1) * P, asl])
                        aT_sb.append(t)
                    if scores_fp32:
                        # f32r copies for the post-softmax path
                        aTr_sb = []
                        for d in range(DT):
                            t = chunk.tile([P, ACH], F32R, tag=f"aTr{d}", name=f"aTr{d}")
                            nc.vector.tensor_copy(t[:], aT_sb[d][:])
                            aTr_sb.append(t)
                    else:
                        aTr_sb = aT_sb

                    # ---- R^T chunk [H, ACH] ----
                    r_sb = []
                    for h in range(HT):
                        ps_r = psum.tile([P, ACH], F32, tag="mm512", name="ps_r")
                        for d in range(DT):
                            nc.tensor.matmul(
                                ps_r[:], wr_sb[d][:, h * P:(h + 1) * P], aT_sb[d][:],
                                start=(d == 0), stop=(d == DT - 1))
                        t = chunk.tile([P, ACH], sdt, tag=f"r{h}", name=f"r{h}")
                        nc.scalar.activation(t[:], ps_r[:], AFT.Copy)
                        r_sb.append(t)

                    # ---- scores + softmax + transpose -> attn^T [T, ACH] ----
                    atT_sb = chunk.tile([T, ACH], F32R, tag="atT", name="atT_sb")
                    for j in range(AT_CH):
                        jsl = slice(j * P, (j + 1) * P)
                        ps_s = psum.tile([P, T], F32, tag="sc", name="ps_s")
                        for h in range(HT):
                            nc.tensor.matmul(ps_s[:], r_sb[h][:, jsl], qt_sb[h][:],
                                             start=(h == 0), stop=False)
                        nc.tensor.matmul(ps_s[:], ones_s[:], mb_sb[:],
                                         start=False, stop=True)
                        nmx = small.tile([P, 1], F32, tag="nmx", name="nmx")
                        nc.vector.reduce_max(out=nmx[:], in_=ps_s[:], axis=AX, negate=True)
                        attn = small.tile([P, T], F32R, tag="attn", name="attn")
                        ssum = small.tile([P, 1], F32, tag="ssum", name="ssum")
                        nc.scalar.activation(attn[:], ps_s[:], AFT.Exp,
                                             bias=nmx[:], scale=1.0, accum_out=ssum[:])
                        rs = small.tile([P, 1], F32, tag="rs", name="rs")
                        nc.vector.reciprocal(rs[:], ssum[:])
                        nc.vector.tensor_scalar_mul(attn[:], attn[:], rs[:])
                        ps_t = psum.tile([T, P], F32R, tag="tr", name="ps_t")
                        nc.tensor.transpose(ps_t[:], attn[:], ident_r[:])
                        nc.vector.tensor_copy(atT_sb[:, jsl], ps_t[:])

                    # ---- attended^T chunk [H, ACH] + product ----
                    at_sb, pr_sb = [], []
                    for h in range(HT):
                        ps_a = psum.tile([P, ACH], F32, tag="mm512", name="ps_a")
                        nc.tensor.matmul(ps_a[:], qn_sb[:, h * P:(h + 1) * P], atT_sb[:],
                                         start=True, stop=True)
                        t = chunk.tile([P, ACH], F32R, tag=f"at{h}", name=f"at{h}")
                        nc.scalar.activation(t[:], ps_a[:], AFT.Copy)
                        at_sb.append(t)
                        pr = chunk.tile([P, ACH], F32R, tag=f"pr{h}", name=f"pr{h}")
                        nc.vector.tensor_mul(pr[:], aTr_sb[h][:], t[:])
                        pr_sb.append(pr)

                    # ---- final: x = relu(combined^T.T @ W_combine + b) ----
                    ctiles = aTr_sb + at_sb + pr_sb
                    for j in range(AT_CH):
                        jsl = slice(j * P, (j + 1) * P)
                        ps_x = psum.tile([P, H], F32, tag="mm512", name="ps_x")
                        for k in range(CT):
                            nc.tensor.matmul(ps_x[:], ctiles[k][:, jsl], wc_sb[k][:],
                                             start=(k == 0), stop=False)
                        nc.tensor.matmul(ps_x[:], ones_r[:], bc_sb[:],
                                         start=False, stop=True)
                        xo = stage.tile([P, H], F32, tag="xo", name="xo")
                        nc.scalar.activation(xo[:], ps_x[:], AFT.Relu)
                        nc.sync.dma_start(out=x[i, c * ACH + j * P: c * ACH + (j + 1) * P, :],
                                          in_=xo[:])
    nc.compile()
    return nc


def _prep(anchor_feats, query_embs, query_mask, W_region, W_query, W_combine, b_combine):
    """Host-side shard + layout prep. Returns the 8 per-core input maps."""
    f = np.float32
    aT = np.ascontiguousarray(
        np.asarray(anchor_feats, dtype=f).reshape(NCORES, PB, A, D).transpose(0, 1, 3, 2))
    qT = np.ascontiguousarray(
        np.asarray(query_embs, dtype=f).reshape(NCORES, PB, T, D).transpose(0, 1, 3, 2))
    mbv = np.where(np.asarray(query_mask).reshape(NCORES, PB, 1, T) > 0,
                   f(0), f(-1e9)).astype(f)
    wr = np.ascontiguousarray(np.asarray(W_region, dtype=f))
    wq = np.ascontiguousarray(np.asarray(W_query, dtype=f))
    wcv = np.ascontiguousarray(np.asarray(W_combine, dtype=f))
    bcv = np.ascontiguousarray(np.asarray(b_combine, dtype=f)).reshape(1, H)
    return [
        {"aT": aT[cid], "qT": qT[cid], "mb": mbv[cid],
         "wr": wr, "wq": wq, "wc": wcv, "bc": bcv}
        for cid in range(NCORES)
    ]


def kernel(anchor_feats, query_embs, query_mask,
           W_region, W_query, W_combine, b_combine):
    if "nc" not in _CACHE:
        _CACHE["nc"] = build()
    nc = _CACHE["nc"]
    in_maps = _prep(anchor_feats, query_embs, query_mask,
                    W_region, W_query, W_combine, b_combine)
    res = run_bass_kernel_spmd(nc, in_maps, core_ids=list(range(NCORES)))
    out = np.empty((B, A, H), dtype=np.float32)
    for cid in range(NCORES):
        out[cid * PB:(cid + 1) * PB] = res.results[cid]["x"]
    return out


# revision 12
# speedup vs baseline: 72.5668x; 72.5668x over previous
"""CrossModalAttentionScorer Trainium2 kernel (Bass/Tile, 8 NeuronCores).

Reference computation (per batch b):
    R = anchor @ W_region            [A, H]
    Q = query  @ W_query             [T, H]
    S = R @ Q.T  (masked over T)     [A, T]
    P = softmax(S, axis=T)
    att = P @ Q                      [A, H]
    out = relu(concat([anchor, att, anchor*att]) @ W_combine + b)   [A, H]

Shapes: B=32, A=1024, T=64, D=H=512.

Sharding: data-parallel over B across 8 cores (4 batches/core), weights
replicated. Host pre-transposes anchor -> [B, D, A] and query -> [B, D, T] so
the contraction dim is the SBUF partition dim everywhere; every intermediate
is kept feature-major ("transposed") on chip, which makes all five matmuls
chain with no on-device input transposes. Only the [A,T] attention matrix is
PE-transposed (tiny). Matmuls run in float32r (TF32-like, ~1.5e-4 rel err,
4x the fp32 rate at N=512); PSUM accumulation is fp32. The query mask and
the combine bias are folded into the PSUM accumulation as rank-1 (K=1)
matmuls, so masking/bias costs no vector work.

scores_fp32=True falls back to full-fp32 matmuls for the pre-softmax path
(anchor/query projections + scores) at ~4x their PE cost, if the f32r
rounding of attention logits ever proves too coarse.
"""
import numpy as np

import concourse.bacc as bacc
import concourse.tile as tile
import concourse.mybir as mybir
from concourse.bass_utils import run_bass_kernel_spmd
from concourse.masks import make_identity

B, A, T, D, H = 32, 1024, 64, 512, 512
NCORES = 8
PB = B // NCORES          # batches per core = 4
P = 128                   # partitions
DT = D // P               # 4 d-tiles
HT = H // P               # 4 h-tiles
CT = 3 * H // P           # 12 c-tiles (concat dim)
ACH = 512                 # a-chunk (moving-dim) size
NCH = A // ACH            # 2 chunks per batch
AT_CH = ACH // P          # 4 a-tiles per chunk

F32 = mybir.dt.float32
F32R = mybir.dt.float32r
AX = mybir.AxisListType.X
AFT = mybir.ActivationFunctionType

_CACHE = {}


def build(scores_fp32: bool = False, reps: int = 1):
    """Build the per-core Bass module (4 batches of the problem).

    reps>1 repeats the whole computation in one NEFF (for timing-by-slope:
    device time per rep = (T(reps=N) - T(reps=1)) / (N-1), which cancels the
    host/RPC dispatch overhead that dwarfs a single ~200us run)."""
    nc = bacc.Bacc(None, target_bir_lowering=False, debug=False)
    sdt = F32 if scores_fp32 else F32R  # dtype of the pre-softmax path

    aT = nc.dram_tensor("aT", [PB, D, A], sdt, kind="ExternalInput")
    qT = nc.dram_tensor("qT", [PB, D, T], sdt, kind="ExternalInput")
    mb = nc.dram_tensor("mb", [PB, 1, T], sdt, kind="ExternalInput")
    wr = nc.dram_tensor("wr", [D, H], sdt, kind="ExternalInput")
    wq = nc.dram_tensor("wq", [D, H], sdt, kind="ExternalInput")
    wc = nc.dram_tensor("wc", [3 * H, H], F32R, kind="ExternalInput")
    bc = nc.dram_tensor("bc", [1, H], F32R, kind="ExternalInput")
    x = nc.dram_tensor("x", [PB, A, H], F32, kind="ExternalOutput")

    with tile.TileContext(nc) as tc:
        with (
            tc.tile_pool(name="const", bufs=1) as const,
            tc.tile_pool(name="perb", bufs=2) as perb,
            tc.tile_pool(name="chunk", bufs=2) as chunk,
            tc.tile_pool(name="small", bufs=4) as small,
            tc.tile_pool(name="stage", bufs=3) as stage,
            tc.tile_pool(name="psum", bufs=4, space="PSUM") as psum,
        ):
            # ---- constants ----
            wr_sb, wq_sb, wc_sb = [], [], []
            for d in range(DT):
                t = const.tile([P, H], sdt, name=f"wq{d}")
                nc.sync.dma_start(out=t, in_=wq[d * P:(d + 1) * P, :])
                wq_sb.append(t)
            for d in range(DT):
                t = const.tile([P, H], sdt, name=f"wr{d}")
                nc.sync.dma_start(out=t, in_=wr[d * P:(d + 1) * P, :])
                wr_sb.append(t)
            for k in range(CT):
                t = const.tile([P, H], F32R, name=f"wc{k}")
                wc_sb.append(t)
            bc_sb = const.tile([1, H], F32R, name="bc_sb")
            nc.sync.dma_start(out=bc_sb, in_=bc[:, :])
            ones_f = const.tile([1, P], F32, name="ones_f")
            nc.vector.memset(ones_f, 1.0)
            ones_r = const.tile([1, P], F32R, name="ones_r")
            nc.vector.tensor_copy(ones_r[:], ones_f[:])
            ones_s = ones_f if scores_fp32 else ones_r
            ident = const.tile([P, P], F32, name="ident")
            make_identity(nc, ident)
            ident_r = const.tile([P, P], F32R, name="ident_r")
            nc.vector.tensor_copy(ident_r[:], ident[:])

            def emit_qphase(i):
                # per-batch query projections (tiny)
                qT_sb = []
                for d in range(DT):
                    t = perb.tile([P, T], sdt, tag=f"qT{d}", name=f"qT{d}")
                    nc.scalar.dma_start(out=t, in_=qT[i, d * P:(d + 1) * P, :])
                    qT_sb.append(t)
                mb_sb = perb.tile([1, T], sdt, tag="mb", name="mb_sb")
                nc.scalar.dma_start(out=mb_sb, in_=mb[i, :, :])

                # Q natural [T, H] (lhsT for attendedT)
                ps_q = psum.tile([T, H], F32, tag="sc", bufs=2, name="ps_q")
                for d in range(DT):
                    nc.tensor.matmul(ps_q[:], qT_sb[d][:], wq_sb[d][:],
                                     start=(d == 0), stop=(d == DT - 1))
                qn_sb = perb.tile([T, H], F32R, tag="qn", name="qn_sb")
                nc.scalar.activation(qn_sb[:], ps_q[:], AFT.Copy)

                # Q^T [H, T] (rhs for scores) via PE transpose of Q natural
                qt_sb = []
                for h in range(HT):
                    ps_qt = psum.tile([P, T], F32R, tag="tr", bufs=2, name="ps_qt")
                    nc.tensor.transpose(ps_qt[:], qn_sb[:, h * P:(h + 1) * P],
                                        ident_r[:T, :T])
                    t = perb.tile([P, T], sdt, tag=f"qt{h}", name=f"qt{h}")
                    nc.vector.tensor_copy(t[:], ps_qt[:])
                    qt_sb.append(t)
                return qn_sb, qt_sb, mb_sb

            pending_final = [None]

            def emit_final(ctiles, i, c):
                def emit():
                    for j in range(AT_CH):
                        jsl = slice(j * P, (j + 1) * P)
                        ps_x = psum.tile([P, H], F32, tag="mm512", name="ps_x")
                        for k in range(CT):
                            nc.tensor.matmul(ps_x[:], ctiles[k][:, jsl], wc_sb[k][:],
                                             start=(k == 0), stop=False)
                        nc.tensor.matmul(ps_x[:], ones_r[:], bc_sb[:],
                                         start=False, stop=True)
                        xo = stage.tile([P, H], F32, tag="xo", name="xo")
                        nc.scalar.activation(xo[:], ps_x[:], AFT.Relu)
                        nc.sync.dma_start(
                            out=x[i, c * ACH + j * P: c * ACH + (j + 1) * P, :],
                            in_=xo[:])
                return emit

            for rep in range(reps):
              for i in range(PB):
                qn_sb, qt_sb, mb_sb = emit_qphase(i)

                for c in range(NCH):
                    asl = slice(c * ACH, (c + 1) * ACH)
                    # ---- anchor^T chunk [D, ACH] ----
                    aT_sb = []
                    for d in range(DT):
                        t = chunk.tile([P, ACH], sdt, tag=f"aT{d}", name=f"aT{d}")
                        nc.sync.dma_start(out=t, in_=aT[i, d * P:(d + 1) * P, asl])
                        aT_sb.append(t)
                    if i == 0 and c == 0 and rep == 0:
                        # Deferred: wc is first needed by the final matmul of
                        # this chunk; loading it after wr/wq/qT/aT lets the R
                        # matmuls start ~10us earlier.
                        for k in range(CT):
                            nc.sync.dma_start(out=wc_sb[k], in_=wc[k * P:(k + 1) * P, :])
                    if scores_fp32:
                        # f32r copies for the post-softmax path
                        aTr_sb = []
                        for d in range(DT):
                            t = chunk.tile([P, ACH], F32R, tag=f"aTr{d}", name=f"aTr{d}")
                            nc.vector.tensor_copy(t[:], aT_sb[d][:])
                            aTr_sb.append(t)
                    else:
                        aTr_sb = aT_sb

                    # ---- R^T chunk [H, ACH] ----
                    r_sb = []
                    for h in range(HT):
                        ps_r = psum.tile([P, ACH], F32, tag="mm512", name="ps_r")
                        for d in range(DT):
                            nc.tensor.matmul(
                                ps_r[:], wr_sb[d][:, h * P:(h + 1) * P], aT_sb[d][:],
                                start=(d == 0), stop=(d == DT - 1))
                        t = chunk.tile([P, ACH], sdt, tag=f"r{h}", name=f"r{h}")
                        nc.scalar.activation(t[:], ps_r[:], AFT.Copy)
                        r_sb.append(t)

                    # ---- scores + softmax (all a-tiles), then transposes ----
                    # Emitting the 4 transpose matmuls after all 4 score
                    # groups lets softmax j overlap score matmuls j+1..3 on
                    # the in-order PE stream instead of stalling it per tile.
                    atT_sb = chunk.tile([T, ACH], F32R, tag="atT", name="atT_sb")
                    attns = []
                    for j in range(AT_CH):
                        jsl = slice(j * P, (j + 1) * P)
                        ps_s = psum.tile([P, T], F32, tag="sc", bufs=2, name="ps_s")
                        for h in range(HT):
                            nc.tensor.matmul(ps_s[:], r_sb[h][:, jsl], qt_sb[h][:],
                                             start=(h == 0), stop=False)
                        nc.tensor.matmul(ps_s[:], ones_s[:], mb_sb[:],
                                         start=False, stop=True)
                        nmx = small.tile([P, 1], F32, tag="nmx", name="nmx")
                        nc.vector.reduce_max(out=nmx[:], in_=ps_s[:], axis=AX, negate=True)
                        attn = small.tile([P, T], F32R, tag="attn", name="attn")
                        ssum = small.tile([P, 1], F32, tag="ssum", name="ssum")
                        nc.scalar.activation(attn[:], ps_s[:], AFT.Exp,
                                             bias=nmx[:], scale=1.0, accum_out=ssum[:])
                        rs = small.tile([P, 1], F32, tag="rs", name="rs")
                        nc.vector.reciprocal(rs[:], ssum[:])
                        nc.vector.tensor_scalar_mul(attn[:], attn[:], rs[:])
                        attns.append(attn)
                    # previous chunk's final matmuls fill the PE stream while
                    # this chunk's softmax runs on ACT/DVE
                    if pending_final[0] is not None:
                        pending_final[0]()
                    for j in range(AT_CH):
                        jsl = slice(j * P, (j + 1) * P)
                        ps_t = psum.tile([T, P], F32R, tag="tr", bufs=2, name="ps_t")
                        nc.tensor.transpose(ps_t[:], attns[j][:], ident_r[:])
                        nc.vector.tensor_copy(atT_sb[:, jsl], ps_t[:])

                    # ---- attended^T chunk [H, ACH] + product ----
                    at_sb, pr_sb = [], []
                    for h in range(HT):
                        ps_a = psum.tile([P, ACH], F32, tag="mm512", name="ps_a")
                        nc.tensor.matmul(ps_a[:], qn_sb[:, h * P:(h + 1) * P], atT_sb[:],
                                         start=True, stop=True)
                        t = chunk.tile([P, ACH], F32R, tag=f"at{h}", name=f"at{h}")
                        nc.scalar.activation(t[:], ps_a[:], AFT.Copy)
                        at_sb.append(t)
                        pr = chunk.tile([P, ACH], F32R, tag=f"pr{h}", name=f"pr{h}")
                        nc.vector.tensor_mul(pr[:], aTr_sb[h][:], t[:])
                        pr_sb.append(pr)

                    # ---- final: x = relu(combined^T.T @ W_combine + b) ----
                    # deferred: emitted after the NEXT chunk's scores
                    pending_final[0] = emit_final(aTr_sb + at_sb + pr_sb, i, c)
            pending_final[0]()
    nc.compile()
    return nc


def _prep(anchor_feats, query_embs, query_mask, W_region, W_query, W_combine, b_combine):
    """Host-side shard + layout prep. Returns the 8 per-core input maps."""
    f = np.float32
    aT = np.ascontiguousarray(
        np.asarray(anchor_feats, dtype=f).reshape(NCORES, PB, A, D).transpose(0, 1, 3, 2))
    qT = np.ascontiguousarray(
        np.asarray(query_embs, dtype=f).reshape(NCORES, PB, T, D).transpose(0, 1, 3, 2))
    mbv = np.where(np.asarray(query_mask).reshape(NCORES, PB, 1, T) > 0,
                   f(0), f(-1e9)).astype(f)
    wr = np.ascontiguousarray(np.asarray(W_region, dtype=f))
    wq = np.ascontiguousarray(np.asarray(W_query, dtype=f))
    wcv = np.ascontiguousarray(np.asarray(W_combine, dtype=f))
    bcv = np.ascontiguousarray(np.asarray(b_combine, dtype=f)).reshape(1, H)
    return [
        {"aT": aT[cid], "qT": qT[cid], "mb": mbv[cid],
         "wr": wr, "wq": wq, "wc": wcv, "bc": bcv}
        for cid in range(NCORES)
    ]


def kernel(anchor_feats, query_embs, query_mask,
           W_region, W_query, W_combine, b_combine):
    if "nc" not in _CACHE:
        _CACHE["nc"] = build()
    nc = _CACHE["nc"]
    in_maps = _prep(anchor_feats, query_embs, query_mask,
                    W_region, W_query, W_combine, b_combine)
    res = run_bass_kernel_spmd(nc, in_maps, core_ids=list(range(NCORES)))
    out = np.empty((B, A, H), dtype=np.float32)
    for cid in range(NCORES):
        out[cid * PB:(cid + 1) * PB] = res.results[cid]["x"]
    return out
